# revision 21
# baseline (speedup 1.0000x reference)
"""Trainium2 Bass kernel for batched Bayesian Knowledge Tracing (BKT).

Problem: B=4096 students x T=512 timesteps, K=2048 skills. Reference runs a
sequential per-timestep gather/update/scatter over a [B, K] mastery state.

Key reformulation: in odds space (lam = p/(1-p)) one BKT step is affine:
    posterior odds:  lam_post = lam * r,  r = (1-s)/g  (correct)  or s/(1-g)
    learn step:      lam' = (lam_post + t)/(1-t) = A*lam + C
with A = r/(1-t), C = t/(1-t). Tracking mu = 1 + lam keeps the output map
cheap (p = 1 - 1/mu) and the recurrence stays affine:
    mu' = A*mu + (1 + C - A)
Per (student, skill) the updates form a chain over that skill's occurrences.
Sorting each student's timesteps by (skill, time) makes every chain a
contiguous run, and a single hardware affine scan (tensor_tensor_scan with
op0=mult, op1=add) evaluates ALL chains in one pass: at each chain start the
multiplier is set to 0 and the addend to mu0 = 1/(1-k0), which resets the
running state to the prior regardless of what came before. The emitted value
at position j must be the PRE-update mastery, so each element carries its
chain-predecessor's coefficients (shifted by one within the chain).

Host side (numpy): per-row argsort by skill, per-element parameter lookup,
coefficient build + shift, and the inverse reorder of the result back to
time order. Device side: the full recurrence (hardware affine scan), the
odds->probability map. Data parallel over 8 NeuronCores: 512 students each.

Per-core layout: 512 students = 4 blocks of 128 partitions; a partition row
holds its 4 students' T=512 segments concatenated ([128, 2048]). Each
512-column chunk is one student block, processed as a pipelined unit (DMA
in -> scan -> reciprocal -> map -> DMA out) so DMA/DVE/ACT overlap. Scans
never leak across chunk boundaries because position 0 of every student's
permuted sequence is a chain start (multiplier 0).
"""

import os
import numpy as np

B, T, K = 4096, 512, 2048
N_CORES = 8
B_CORE = B // N_CORES        # 512 students per core
NBLK = B_CORE // 128         # 4 partition blocks
FREE = NBLK * T              # 2048 free-dim elements per partition

_prog_cache = {}


def _build_program(W):
    """W = packed chain-region width (columns per student that need the scan).

    Per-chunk input layout: [data0_packed (W) | data1 (T)]. The scan runs
    in-place over data1's first W columns (out == data1 region: elementwise
    stream, read precedes write per element). Columns [W, T) of data1 belong
    to singleton chains where data0 = 0, so mu = data1 there already -- no
    scan needed.
    """
    if W in _prog_cache:
        return _prog_cache[W]

    import concourse.bacc as bacc
    import concourse.tile as tile
    import concourse.mybir as mybir

    nc = bacc.Bacc(
        "TRN2",
        target_bir_lowering=False,
        debug=False,
        num_devices=N_CORES,
    )
    f32 = mybir.dt.float32
    C = W + T  # columns per chunk
    # dense per-chunk blocks: row stride == transfer width, so each sub-DMA
    # reads one packed linear region
    dinA = nc.dram_tensor("dataA", [NBLK * 128, 2 * W], f32, kind="ExternalInput")
    dinB = nc.dram_tensor("dataB", [NBLK * 128, C - 2 * W], f32, kind="ExternalInput")
    out = nc.dram_tensor("out", [128, FREE], f32, kind="ExternalOutput")

    with tile.TileContext(nc) as tc:
        with tc.tile_pool(name="main", bufs=1) as pool:
            # Per-queue HWDGE throughput is ~150-200 GB/s; two queues (SP,
            # ACT) halve each chunk's arrival time. All triggers are emitted
            # before any compute so transfers start immediately.
            # Scan inputs are exactly [0, 2W); splitting there lets each scan
            # depend only on its A-part. A-parts alternate over the two HWDGE
            # queues (SP/ACT) so consecutive chunks arrive in parallel; the
            # first two B-parts (singleton region, needed later by recip) ride
            # the GPSIMD SWDGE queue as a third channel.
            ins = []
            for b in range(NBLK):
                ins.append(pool.tile([128, C], f32, tag=f"in{b}", name=f"in{b}"))
            for b in range(NBLK):
                eng = nc.sync if b % 2 == 0 else nc.scalar
                eng.dma_start(
                    ins[b][:, :2 * W], dinA.ap()[b * 128:(b + 1) * 128, :]
                )
            for b in range(NBLK):
                eng = nc.scalar if b % 2 == 0 else nc.sync
                eng.dma_start(
                    ins[b][:, 2 * W:], dinB.ap()[b * 128:(b + 1) * 128, :]
                )
            for b in range(NBLK):
                s = ins[b]
                # mu[j] = data0[j]*mu[j-1] + data1[j]  (fp32 state), only over
                # the packed chain region; in-place into the data1 columns
                nc.vector.tensor_tensor_scan(
                    s[:, W:2 * W], s[:, :W], s[:, W:2 * W], 0.0,
                    mybir.AluOpType.mult, mybir.AluOpType.add,
                )
                # p = 1 - 1/mu  (mu >= 1.01 always, approx recip is safe)
                r = pool.tile([128, T], f32, tag=f"r{b}")
                nc.vector.reciprocal_approx_fast(r[:], s[:, W:C])
                p = pool.tile([128, T], f32, tag=f"p{b}")
                nc.scalar.activation(
                    p[:], r[:], mybir.ActivationFunctionType.Copy,
                    bias=1.0, scale=-1.0,
                )
                eng = nc.sync if b % 2 == 0 else nc.scalar
                if b < NBLK - 1:
                    eng.dma_start(out.ap()[:, b * T:(b + 1) * T], p[:])
                else:
                    # split the last store so the kernel tail is short
                    hh = T // 2
                    nc.sync.dma_start(out.ap()[:, b * T:b * T + hh], p[:, :hh])
                    nc.scalar.dma_start(
                        out.ap()[:, b * T + hh:(b + 1) * T], p[:, hh:]
                    )

    nc.compile()
    _prog_cache[W] = nc
    return nc


def _prepare(skills, responses, k0, t, g, s):
    """Host preprocessing: permutation, parameter lookup, scan coefficients."""
    f32 = np.float32
    one = f32(1.0)
    perm = np.argsort(skills, axis=1, kind="stable")        # [B,T]
    sk_p = np.take_along_axis(skills, perm, 1)
    res_p = np.take_along_axis(responses, perm, 1)
    start = np.ones((B, T), dtype=bool)
    start[:, 1:] = sk_p[:, 1:] != sk_p[:, :-1]

    tt = t[sk_p].astype(f32)
    lr = np.where(
        res_p == 1.0,
        (one - s[sk_p].astype(f32)) / g[sk_p].astype(f32),
        s[sk_p].astype(f32) / (one - g[sk_p].astype(f32)),
    ).astype(f32)
    A = (lr / (one - tt)).astype(f32)                       # mult coeff
    D1 = (one + tt / (one - tt) - A).astype(f32)            # addend (mu form)
    mu0 = (one / (one - k0.astype(f32)))[sk_p]              # reset value

    data0 = np.zeros((B, T), f32)
    data1 = np.empty((B, T), f32)
    data0[:, 1:] = np.where(start[:, 1:], f32(0), A[:, :-1])
    data1[:, 0] = mu0[:, 0]
    data1[:, 1:] = np.where(start[:, 1:], mu0[:, 1:], D1[:, :-1])

    # Pack multi-occurrence chains (run length >= 2) to the front of each
    # row; singletons (mu = data1 directly, no recurrence) go last. Chains
    # keep their relative order, so the shifted coefficients stay aligned.
    rid = np.cumsum(start, axis=1)                          # run id, 1-based
    row_off = (np.arange(B) * (T + 1))[:, None]
    counts = np.bincount((rid + row_off).ravel(), minlength=B * (T + 1))
    run_len = counts.reshape(B, T + 1)[
        np.arange(B)[:, None], rid
    ]
    multi = run_len >= 2
    order2 = np.argsort(~multi, axis=1, kind="stable")      # multi first
    data0 = np.take_along_axis(data0, order2, 1)
    data1 = np.take_along_axis(data1, order2, 1)
    perm2 = np.take_along_axis(perm, order2, 1)

    # W = scan width: max packed-chain columns over all rows, padded up
    W = int(multi.sum(axis=1).max())
    W = min(T, (W + 63) & ~63)
    return data0[:, :W], data1, perm2, W


def _core_layout(plane, c):
    """[B,T]-like plane -> this core's [128, NBLK*width] SBUF-shaped array."""
    w = plane.shape[1]
    chunk = plane[c * B_CORE:(c + 1) * B_CORE]
    return np.ascontiguousarray(
        chunk.reshape(NBLK, 128, w).transpose(1, 0, 2).reshape(128, NBLK * w)
    )


def _ensure_ntff_hook():
    """The agent image's antenv lacks axon_hooks; shim it so trace=True can
    register the ctypes NTFF profiler from trn_agent_boot. Test-only path."""
    import sys, types
    try:
        from antenv import axon_hooks  # noqa: F401
        return
    except ImportError:
        pass
    mod = types.ModuleType("antenv.axon_hooks")
    holder = [None]
    mod.get_axon_ntff_profile_hook = lambda: holder[0]
    mod.set_axon_ntff_profile_hook = lambda h: holder.__setitem__(0, h)
    sys.modules["antenv.axon_hooks"] = mod
    import antenv
    antenv.axon_hooks = mod
    try:
        from trn_agent_boot.trn_boot import _ntff_profile_via_ctypes
        mod.set_axon_ntff_profile_hook(
            _ntff_profile_via_ctypes("/opt/axon/libaxon_pjrt.so")
        )
    except Exception as e:  # degrade to untraced run
        print(f"NTFF hook unavailable: {e}")


def kernel(skills, responses, k0, t, g, s, num_skills=None, **_unused):
    skills = np.asarray(skills)
    responses = np.asarray(responses, dtype=np.float32)
    k0 = np.asarray(k0, dtype=np.float32)
    t = np.asarray(t, dtype=np.float32)
    g = np.asarray(g, dtype=np.float32)
    s = np.asarray(s, dtype=np.float32)
    assert skills.shape == (B, T) and responses.shape == (B, T)

    data0p, data1, perm, W = _prepare(skills, responses, k0, t, g, s)

    nc = _build_program(W)
    in_maps = []
    for c in range(N_CORES):
        stu = slice(c * B_CORE, (c + 1) * B_CORE)
        in_maps.append({
            "dataA": np.ascontiguousarray(
                np.concatenate([data0p[stu], data1[stu, :W]], axis=1)
            ),
            "dataB": np.ascontiguousarray(data1[stu, W:]),
        })

    from concourse.bass_utils import run_bass_kernel_spmd

    trace = bool(int(os.environ.get("BKT_TRACE", "0")))
    if trace:
        _ensure_ntff_hook()
    res = run_bass_kernel_spmd(nc, in_maps, list(range(N_CORES)), trace=trace)
    if trace and res.exec_time_ns is not None:
        print(f"HW exec time: {res.exec_time_ns} ns")
        kernel.last_exec_time_ns = res.exec_time_ns

    # gather per-core results (still in permuted order), then undo the sort
    p_perm = np.empty((B, T), np.float32)
    for c in range(N_CORES):
        oc = res.results[c]["out"]
        p_perm[c * B_CORE:(c + 1) * B_CORE] = (
            oc.reshape(128, NBLK, T).transpose(1, 0, 2).reshape(B_CORE, T)
        )
    out = np.empty((B, T), np.float32)
    np.put_along_axis(out, perm, p_perm, axis=1)
    return out


# revision 23
# speedup vs baseline: 1.0263x; 1.0263x over previous
"""Trainium2 Bass kernel for batched Bayesian Knowledge Tracing (BKT).

Problem: B=4096 students x T=512 timesteps, K=2048 skills. Reference runs a
sequential per-timestep gather/update/scatter over a [B, K] mastery state.

Key reformulation: in odds space (lam = p/(1-p)) one BKT step is affine:
    posterior odds:  lam_post = lam * r,  r = (1-s)/g  (correct)  or s/(1-g)
    learn step:      lam' = (lam_post + t)/(1-t) = A*lam + C
with A = r/(1-t), C = t/(1-t). Tracking mu = 1 + lam keeps the output map
cheap (p = 1 - 1/mu) and the recurrence stays affine:
    mu' = A*mu + (1 + C - A)
Per (student, skill) the updates form a chain over that skill's occurrences.
Sorting each student's timesteps by (skill, time) makes every chain a
contiguous run, and a single hardware affine scan (tensor_tensor_scan with
op0=mult, op1=add) evaluates ALL chains in one pass: at each chain start the
multiplier is set to 0 and the addend to mu0 = 1/(1-k0), which resets the
running state to the prior regardless of what came before. The emitted value
at position j must be the PRE-update mastery, so each element carries its
chain-predecessor's coefficients (shifted by one within the chain).

Host side (numpy): per-row argsort by skill, per-element parameter lookup,
coefficient build + shift, and the inverse reorder of the result back to
time order. Device side: the full recurrence (hardware affine scan), the
odds->probability map. Data parallel over 8 NeuronCores: 512 students each.

Per-core layout: 512 students = 4 blocks of 128 partitions; a partition row
holds its 4 students' T=512 segments concatenated ([128, 2048]). Each
512-column chunk is one student block, processed as a pipelined unit (DMA
in -> scan -> reciprocal -> map -> DMA out) so DMA/DVE/ACT overlap. Scans
never leak across chunk boundaries because position 0 of every student's
permuted sequence is a chain start (multiplier 0).
"""

import os
import numpy as np

B, T, K = 4096, 512, 2048
N_CORES = 8
B_CORE = B // N_CORES        # 512 students per core
NBLK = B_CORE // 128         # 4 partition blocks
FREE = NBLK * T              # 2048 free-dim elements per partition

_prog_cache = {}


def _build_program(W):
    """W = packed chain-region width (columns per student that need the scan).

    Per-chunk input layout: [data0_packed (W) | data1 (T)]. The scan runs
    in-place over data1's first W columns (out == data1 region: elementwise
    stream, read precedes write per element). Columns [W, T) of data1 belong
    to singleton chains where data0 = 0, so mu = data1 there already -- no
    scan needed.
    """
    if W in _prog_cache:
        return _prog_cache[W]

    import concourse.bacc as bacc
    import concourse.tile as tile
    import concourse.mybir as mybir

    nc = bacc.Bacc(
        "TRN2",
        target_bir_lowering=False,
        debug=False,
        num_devices=N_CORES,
    )
    f32 = mybir.dt.float32
    C = W + T  # columns per chunk
    din = nc.dram_tensor("data", [128, NBLK * C], f32, kind="ExternalInput")
    out = nc.dram_tensor("out", [128, FREE], f32, kind="ExternalOutput")

    with tile.TileContext(nc) as tc:
        with tc.tile_pool(name="main", bufs=1) as pool:
            # Per-queue HWDGE throughput is ~150-200 GB/s; two queues (SP,
            # ACT) halve each chunk's arrival time. All triggers are emitted
            # before any compute so transfers start immediately.
            # Scan inputs are exactly [0, 2W); splitting there lets each scan
            # depend only on its A-part. A-parts alternate over the two HWDGE
            # queues (SP/ACT) so consecutive chunks arrive in parallel; the
            # first two B-parts (singleton region, needed later by recip) ride
            # the GPSIMD SWDGE queue as a third channel.
            ins = []
            for b in range(NBLK):
                ins.append(pool.tile([128, C], f32, tag=f"in{b}", name=f"in{b}"))
            for b in range(NBLK):
                eng = nc.sync if b % 2 == 0 else nc.scalar
                eng.dma_start(ins[b][:, :2 * W], din.ap()[:, b * C:b * C + 2 * W])
            for b in range(NBLK):
                eng = nc.scalar if b % 2 == 0 else nc.sync
                eng.dma_start(
                    ins[b][:, 2 * W:], din.ap()[:, b * C + 2 * W:(b + 1) * C]
                )
            for b in range(NBLK):
                s = ins[b]
                # mu[j] = data0[j]*mu[j-1] + data1[j]  (fp32 state), only over
                # the packed chain region; in-place into the data1 columns
                nc.vector.tensor_tensor_scan(
                    s[:, W:2 * W], s[:, :W], s[:, W:2 * W], 0.0,
                    mybir.AluOpType.mult, mybir.AluOpType.add,
                )
                # p = 1 - 1/mu  (mu >= 1.01 always, approx recip is safe)
                r = pool.tile([128, T], f32, tag=f"r{b}")
                nc.vector.reciprocal_approx_fast(r[:], s[:, W:C])
                p = pool.tile([128, T], f32, tag=f"p{b}")
                nc.scalar.activation(
                    p[:], r[:], mybir.ActivationFunctionType.Copy,
                    bias=1.0, scale=-1.0,
                )
                eng = nc.sync if b % 2 == 0 else nc.scalar
                if b < NBLK - 1:
                    eng.dma_start(out.ap()[:, b * T:(b + 1) * T], p[:])
                else:
                    # split the last store so the kernel tail is short
                    hh = T // 2
                    nc.sync.dma_start(out.ap()[:, b * T:b * T + hh], p[:, :hh])
                    nc.scalar.dma_start(
                        out.ap()[:, b * T + hh:(b + 1) * T], p[:, hh:]
                    )

    nc.compile()
    _prog_cache[W] = nc
    return nc


def _prepare(skills, responses, k0, t, g, s):
    """Host preprocessing: permutation, parameter lookup, scan coefficients."""
    f32 = np.float32
    one = f32(1.0)
    perm = np.argsort(skills, axis=1, kind="stable")        # [B,T]
    sk_p = np.take_along_axis(skills, perm, 1)
    res_p = np.take_along_axis(responses, perm, 1)
    start = np.ones((B, T), dtype=bool)
    start[:, 1:] = sk_p[:, 1:] != sk_p[:, :-1]

    tt = t[sk_p].astype(f32)
    lr = np.where(
        res_p == 1.0,
        (one - s[sk_p].astype(f32)) / g[sk_p].astype(f32),
        s[sk_p].astype(f32) / (one - g[sk_p].astype(f32)),
    ).astype(f32)
    A = (lr / (one - tt)).astype(f32)                       # mult coeff
    D1 = (one + tt / (one - tt) - A).astype(f32)            # addend (mu form)
    mu0 = (one / (one - k0.astype(f32)))[sk_p]              # reset value

    data0 = np.zeros((B, T), f32)
    data1 = np.empty((B, T), f32)
    data0[:, 1:] = np.where(start[:, 1:], f32(0), A[:, :-1])
    data1[:, 0] = mu0[:, 0]
    data1[:, 1:] = np.where(start[:, 1:], mu0[:, 1:], D1[:, :-1])

    # Pack multi-occurrence chains (run length >= 2) to the front of each
    # row; singletons (mu = data1 directly, no recurrence) go last. Chains
    # keep their relative order, so the shifted coefficients stay aligned.
    rid = np.cumsum(start, axis=1)                          # run id, 1-based
    row_off = (np.arange(B) * (T + 1))[:, None]
    counts = np.bincount((rid + row_off).ravel(), minlength=B * (T + 1))
    run_len = counts.reshape(B, T + 1)[
        np.arange(B)[:, None], rid
    ]
    multi = run_len >= 2
    order2 = np.argsort(~multi, axis=1, kind="stable")      # multi first
    data0 = np.take_along_axis(data0, order2, 1)
    data1 = np.take_along_axis(data1, order2, 1)
    perm2 = np.take_along_axis(perm, order2, 1)

    # W = scan width: max packed-chain columns over all rows, padded up
    W = int(multi.sum(axis=1).max())
    W = min(T, (W + 31) & ~31)
    return data0[:, :W], data1, perm2, W


def _core_layout(plane, c):
    """[B,T]-like plane -> this core's [128, NBLK*width] SBUF-shaped array."""
    w = plane.shape[1]
    chunk = plane[c * B_CORE:(c + 1) * B_CORE]
    return np.ascontiguousarray(
        chunk.reshape(NBLK, 128, w).transpose(1, 0, 2).reshape(128, NBLK * w)
    )


def _ensure_ntff_hook():
    """The agent image's antenv lacks axon_hooks; shim it so trace=True can
    register the ctypes NTFF profiler from trn_agent_boot. Test-only path."""
    import sys, types
    try:
        from antenv import axon_hooks  # noqa: F401
        return
    except ImportError:
        pass
    mod = types.ModuleType("antenv.axon_hooks")
    holder = [None]
    mod.get_axon_ntff_profile_hook = lambda: holder[0]
    mod.set_axon_ntff_profile_hook = lambda h: holder.__setitem__(0, h)
    sys.modules["antenv.axon_hooks"] = mod
    import antenv
    antenv.axon_hooks = mod
    try:
        from trn_agent_boot.trn_boot import _ntff_profile_via_ctypes
        mod.set_axon_ntff_profile_hook(
            _ntff_profile_via_ctypes("/opt/axon/libaxon_pjrt.so")
        )
    except Exception as e:  # degrade to untraced run
        print(f"NTFF hook unavailable: {e}")


def kernel(skills, responses, k0, t, g, s, num_skills=None, **_unused):
    skills = np.asarray(skills)
    responses = np.asarray(responses, dtype=np.float32)
    k0 = np.asarray(k0, dtype=np.float32)
    t = np.asarray(t, dtype=np.float32)
    g = np.asarray(g, dtype=np.float32)
    s = np.asarray(s, dtype=np.float32)
    assert skills.shape == (B, T) and responses.shape == (B, T)

    data0p, data1, perm, W = _prepare(skills, responses, k0, t, g, s)

    nc = _build_program(W)
    # per-row chunk layout [d0_packed (W) | d1_chains (W) | d1_singles]
    merged = np.concatenate([data0p, data1[:, :W], data1[:, W:]], axis=1)
    in_maps = [{"data": _core_layout(merged, c)} for c in range(N_CORES)]

    from concourse.bass_utils import run_bass_kernel_spmd

    trace = bool(int(os.environ.get("BKT_TRACE", "0")))
    if trace:
        _ensure_ntff_hook()
    res = run_bass_kernel_spmd(nc, in_maps, list(range(N_CORES)), trace=trace)
    if trace and res.exec_time_ns is not None:
        print(f"HW exec time: {res.exec_time_ns} ns")
        kernel.last_exec_time_ns = res.exec_time_ns

    # gather per-core results (still in permuted order), then undo the sort
    p_perm = np.empty((B, T), np.float32)
    for c in range(N_CORES):
        oc = res.results[c]["out"]
        p_perm[c * B_CORE:(c + 1) * B_CORE] = (
            oc.reshape(128, NBLK, T).transpose(1, 0, 2).reshape(B_CORE, T)
        )
    out = np.empty((B, T), np.float32)
    np.put_along_axis(out, perm, p_perm, axis=1)
    return out
